# revision 2
# baseline (speedup 1.0000x reference)
"""Trainium2 Bass kernel for nn_LogicLayer (differentiable logic-gate layer).

Reference computation:
    a = x[:, idx_a]; b = x[:, idx_b]                  # [B, OUT] gathers
    w = softmax(weights, -1)                          # [OUT, 16]
    out = sum_k w[:, k] * gate_k(a, b)

Every gate value is of the form c0 + c1*a + c2*b + c3*a*b, so
    out[i, j] = W0[j] + W1[j]*a + W2[j]*b + W3[j]*a*b
with W = softmax(weights) @ C, C the [16, 4] gate-coefficient table
(host-precomputed metadata, like the index packing).

Sharding: out_dim split 8 ways (1024 j's per core), batch unsharded.
Each core receives x pre-transposed to fp16 xT [IN, B] in DRAM, so the
kernel is a single gather->gates->transpose-back phase, 2D-pipelined
over (j-chunk, batch-half):
  1. dma_gather fp16 rows of xT (elem = one batch half) for idx_a/idx_b
     (j lands on partitions in (q, r) wrap: j = r*128 + q)
  2. u = W3*a + W2 (ACT), v = W1*a + W0 (DVE ts), t = u*b (DVE tt)
  3. out = t + v realized in PSUM fp16 by PE transpose-accumulate,
     copied to SBUF (DVE/ACT/Pool round-robin) and stored batched
     (4 h-blocks per DMA) in natural [B, OUT] layout.
Output is stored fp16 (well within the 2e-2 gate tolerance since all
gate math is fp16 anyway) and upcast to f32 on the host.
"""

import numpy as np

# ---------------------------------------------------------------- constants
B_TOT, IN_DIM, OUT_DIM = 2048, 8192, 8192
NCORES = 8
OUT_PC = OUT_DIM // NCORES   # 1024 j's per core

# value = c0 + c1*a + c2*b + c3*ab  for each of the 16 gates
GATE_C = np.array(
    [
        # c0  c1  c2  c3
        [0, 0, 0, 0],    # 0  False
        [0, 0, 0, 1],    # 1  a AND b
        [0, 1, 0, -1],   # 2  a AND NOT b
        [0, 1, 0, 0],    # 3  a
        [0, 0, 1, -1],   # 4  NOT a AND b
        [0, 0, 1, 0],    # 5  b
        [0, 1, 1, -2],   # 6  a XOR b
        [0, 1, 1, -1],   # 7  a OR b
        [1, -1, -1, 1],  # 8  NOT (a OR b)
        [1, -1, -1, 2],  # 9  NOT (a XOR b)
        [1, 0, -1, 0],   # 10 NOT b
        [1, 0, -1, 1],   # 11 a OR NOT b
        [1, -1, 0, 0],   # 12 NOT a
        [1, -1, 0, 1],   # 13 NOT a OR b
        [1, 0, 0, -1],   # 14 NOT (a AND b)
        [1, 0, 0, 0],    # 15 True
    ],
    dtype=np.float32,
)  # [16, 4]


# ---------------------------------------------------------------- device IR
def build_nc(B=B_TOT, IN=IN_DIM, OUT=OUT_PC, NJ=256, Q=2):
    """Build the per-core Bass module (SPMD; all cores run the same IR)."""
    import sys

    if "/opt/trn_rl_repo" not in sys.path:
        sys.path.insert(0, "/opt/trn_rl_repo")

    import concourse.tile as tile
    from concourse import bacc, mybir
    from concourse.masks import make_identity
    from contextlib import ExitStack

    f32 = mybir.dt.float32
    f16 = mybir.dt.float16
    i16 = mybir.dt.int16
    BH = B // Q              # batch half (1024)
    PBH = BH // 128          # h-blocks per half (8)
    NCH = OUT // NJ          # j chunks per core (4)
    SLOTS = NJ // 128        # 128-wide j slots per chunk (2)
    RPT = OUT // 128         # j slots per core (8)
    HG = 4                   # h-blocks batched per PSUM bank / out store
    IW = OUT // 16           # idx columns per tensor (64)
    MW = 2 * IW + 8 * RPT    # meta width in i16: idxa | idxb | wk-bits

    nc = bacc.Bacc("TRN2", target_bir_lowering=False)
    xT = nc.declare_dram_parameter("xT", [IN, B], f16, isOutput=False)
    meta = nc.declare_dram_parameter("meta16", [128, MW], i16, isOutput=False)
    out = nc.declare_dram_parameter("out", [B, OUT], f16, isOutput=True)

    Ident = mybir.ActivationFunctionType.Identity
    MULT = mybir.AluOpType.mult
    ADD = mybir.AluOpType.add

    with tile.TileContext(nc) as tc, ExitStack() as ctx:
        cpool = ctx.enter_context(tc.tile_pool(name="consts", bufs=1))
        mt = cpool.tile([128, MW], i16, name="mt")
        nc.sync.dma_start(mt[:], meta[:])
        wk = cpool.tile([128, 4 * RPT], f32, name="wk")
        nc.vector.tensor_copy(wk[:], mt[:, 2 * IW:MW].bitcast(f32))
        identf = cpool.tile([128, 128], f32, name="identf")
        make_identity(nc, identf[:])
        ident = cpool.tile([128, 128], f16, name="ident")
        nc.vector.tensor_copy(ident[:], identf[:])

        gpool = ctx.enter_context(tc.tile_pool(name="gath", bufs=6))
        upool = ctx.enter_context(tc.tile_pool(name="uvt", bufs=6))
        psumO = ctx.enter_context(tc.tile_pool(name="psumO", bufs=4, space="PSUM"))
        ostg = ctx.enter_context(tc.tile_pool(name="ostg", bufs=6))

        NJ16 = NJ // 16
        # issue every gather up front so DMA engines stay saturated
        gab = {}
        for ck in range(NCH):
            for q in range(Q):
                ga = gpool.tile([128, SLOTS, BH], f16, tag="ga")
                nc.gpsimd.dma_gather(
                    ga[:], xT[:, q * BH:(q + 1) * BH],
                    mt[:, ck * NJ16:(ck + 1) * NJ16],
                    NJ, NJ, BH, elem_step=B,
                )
                gb = gpool.tile([128, SLOTS, BH], f16, tag="gb")
                nc.gpsimd.dma_gather(
                    gb[:], xT[:, q * BH:(q + 1) * BH],
                    mt[:, IW + ck * NJ16:IW + (ck + 1) * NJ16],
                    NJ, NJ, BH, elem_step=B,
                )
                gab[ck, q] = (ga, gb)

        ncp = 0  # copy-engine round robin counter

        def emit_drain(stage):
            """PE transpose-accumulate + PSUM drain + store for one stage.

            The transposes are REAL matmuls against the fp16 identity
            (lhsT.T @ I): fp16 is_transpose matmuls do not accumulate in
            PSUM on hardware, but ordinary fp16 matmuls accumulate in f32
            PSUM at the same 1 cycle/row.
            """
            nonlocal ncp
            ck, q, ts_t, ts_v = stage
            for hg in range(PBH // HG):
                og = ostg.tile([128, HG, NJ], f16, tag="og")
                po = psumO.tile([128, HG, NJ], f32, tag="po")
                for hi in range(HG):
                    hs = slice((hg * HG + hi) * 128, (hg * HG + hi + 1) * 128)
                    for c in range(SLOTS):
                        nc.tensor.matmul(
                            po[:, hi, c * 128:(c + 1) * 128],
                            ts_t[c][:, hs], ident[:],
                            start=True, stop=False,
                        )
                        nc.tensor.matmul(
                            po[:, hi, c * 128:(c + 1) * 128],
                            ts_v[c][:, hs], ident[:],
                            start=False, stop=True,
                        )
                # round-robin the PSUM drain (Pool cannot read PSUM on HW)
                eng = (nc.vector, nc.scalar, nc.vector)[ncp % 3]
                ncp += 1
                if eng is nc.scalar:
                    eng.copy(og[:], po[:])
                else:
                    eng.tensor_copy(og[:], po[:])
                row0 = q * BH + hg * HG * 128
                nc.sync.dma_start(
                    out[row0:row0 + HG * 128, ck * NJ:(ck + 1) * NJ]
                    .rearrange("(i p) j -> p i j", p=128),
                    og[:],
                )

        # software-pipelined emission: stage k's drain section is emitted
        # after stage k+1's elementwise section so the PSUM-drain copies
        # don't head-of-line-block the next stage's v/t in the engine queues
        pending = None
        for ck in range(NCH):
            for q in range(Q):
                ga, gb = gab[ck, q]
                ts_v, ts_t = [], []
                for c in range(SLOTS):
                    r = ck * SLOTS + c
                    # u = W3*a + W2 on ACT (per-partition scalar scale/bias)
                    u = upool.tile([128, BH], f16, tag="u")
                    nc.scalar.activation(
                        u[:], ga[:, c, :], Ident,
                        scale=wk[:, 3 * RPT + r:3 * RPT + r + 1],
                        bias=wk[:, 2 * RPT + r:2 * RPT + r + 1],
                    )
                    # v = W1*a + W0 on DVE (all-fp16 -> 2x DVE mode)
                    v = upool.tile([128, BH], f16, tag="v")
                    nc.vector.tensor_scalar(
                        v[:], ga[:, c, :],
                        wk[:, 1 * RPT + r:1 * RPT + r + 1],
                        wk[:, 0 * RPT + r:0 * RPT + r + 1],
                        op0=MULT, op1=ADD,
                    )
                    # t = u * b on DVE (fp16 2x)
                    t = upool.tile([128, BH], f16, tag="t")
                    nc.vector.tensor_tensor(t[:], u[:], gb[:, c, :], op=MULT)
                    ts_v.append(v)
                    ts_t.append(t)
                if pending is not None:
                    emit_drain(pending)
                pending = (ck, q, ts_t, ts_v)
        emit_drain(pending)
    nc.compile()
    return nc


# ---------------------------------------------------------------- host side
def _wrap_idx(idx, OUT, NJ):
    """Pack an index vector into dma_gather's wrapped int16 layout.

    Per chunk ck the NJ indices live in columns [ck*NJ/16, (ck+1)*NJ/16):
    idx16[p, ck*NJ/16 + s] = idx[ck*NJ + s*16 + p%16], replicated over the
    8 groups of 16 partitions.
    """
    nch = OUT // NJ
    a = np.asarray(idx).astype(np.int16).reshape(nch, NJ // 16, 16)  # [ck, s, p]
    a = a.transpose(2, 0, 1).reshape(16, nch * (NJ // 16))           # [p, ck*s]
    return np.ascontiguousarray(np.tile(a, (8, 1)))                  # [128, ...]


def _prep_inputs(x, weights, idx_a, idx_b, NJ=256):
    x = np.asarray(x, dtype=np.float32)
    weights = np.asarray(weights, dtype=np.float32)
    # W = softmax(weights) @ C  -> [OUT, 4]
    e = np.exp(weights - weights.max(axis=-1, keepdims=True))
    W = (e / e.sum(axis=-1, keepdims=True)).astype(np.float32) @ GATE_C
    xT = np.ascontiguousarray(x.T.astype(np.float16))  # [IN, B], replicated
    RPT = OUT_PC // 128
    in_maps = []
    for core in range(NCORES):
        j0 = core * OUT_PC
        # wk4[q, m*RPT + r] = W[j0 + r*128 + q, m]
        Wc = W[j0:j0 + OUT_PC].reshape(RPT, 128, 4)       # [r, q, m]
        wk4 = np.ascontiguousarray(
            Wc.transpose(1, 2, 0).reshape(128, 4 * RPT)   # [q, m*RPT+r]
        )
        meta = np.concatenate(
            [
                _wrap_idx(idx_a[j0:j0 + OUT_PC], OUT_PC, NJ),
                _wrap_idx(idx_b[j0:j0 + OUT_PC], OUT_PC, NJ),
                wk4.view(np.int16),
            ],
            axis=1,
        )
        in_maps.append({"xT": xT, "meta16": np.ascontiguousarray(meta)})
    return in_maps


_NC_CACHE = {}


def _get_nc():
    if "nc" not in _NC_CACHE:
        _NC_CACHE["nc"] = build_nc()
    return _NC_CACHE["nc"]


def kernel(x, weights, idx_a, idx_b):
    import sys

    if "/opt/trn_rl_repo" not in sys.path:
        sys.path.insert(0, "/opt/trn_rl_repo")
    from concourse.bass_utils import run_bass_kernel_spmd

    nc = _get_nc()
    in_maps = _prep_inputs(x, weights, idx_a, idx_b)
    res = run_bass_kernel_spmd(nc, in_maps, list(range(NCORES)))
    return np.concatenate(
        [r["out"] for r in res.results], axis=1, dtype=np.float32
    )


if __name__ == "__main__":
    nc = build_nc()
    print("built OK")


# revision 3
# speedup vs baseline: 1.0667x; 1.0667x over previous
"""Trainium2 Bass kernel for nn_LogicLayer (differentiable logic-gate layer).

Reference computation:
    a = x[:, idx_a]; b = x[:, idx_b]                  # [B, OUT] gathers
    w = softmax(weights, -1)                          # [OUT, 16]
    out = sum_k w[:, k] * gate_k(a, b)

Every gate value is of the form c0 + c1*a + c2*b + c3*a*b, so
    out[i, j] = W0[j] + W1[j]*a + W2[j]*b + W3[j]*a*b
with W = softmax(weights) @ C, C the [16, 4] gate-coefficient table
(host-precomputed metadata, like the index packing).

Sharding: out_dim split 8 ways (1024 j's per core), batch unsharded.
Each core receives x pre-transposed to fp16 xT [IN, B] in DRAM, so the
kernel is a single gather->gates->transpose-back phase, 2D-pipelined
over (j-chunk, batch-half):
  1. dma_gather fp16 rows of xT (elem = one batch half) for idx_a/idx_b
     (j lands on partitions in (q, r) wrap: j = r*128 + q)
  2. u = W3*a + W2 (ACT), v = W1*a + W0 (DVE ts), t = u*b (DVE tt)
  3. out = t + v realized in PSUM fp16 by PE transpose-accumulate,
     copied to SBUF (DVE/ACT/Pool round-robin) and stored batched
     (4 h-blocks per DMA) in natural [B, OUT] layout.
Output is stored fp16 (well within the 2e-2 gate tolerance since all
gate math is fp16 anyway) and upcast to f32 on the host.
"""

import numpy as np

# ---------------------------------------------------------------- constants
B_TOT, IN_DIM, OUT_DIM = 2048, 8192, 8192
NCORES = 8
OUT_PC = OUT_DIM // NCORES   # 1024 j's per core

# value = c0 + c1*a + c2*b + c3*ab  for each of the 16 gates
GATE_C = np.array(
    [
        # c0  c1  c2  c3
        [0, 0, 0, 0],    # 0  False
        [0, 0, 0, 1],    # 1  a AND b
        [0, 1, 0, -1],   # 2  a AND NOT b
        [0, 1, 0, 0],    # 3  a
        [0, 0, 1, -1],   # 4  NOT a AND b
        [0, 0, 1, 0],    # 5  b
        [0, 1, 1, -2],   # 6  a XOR b
        [0, 1, 1, -1],   # 7  a OR b
        [1, -1, -1, 1],  # 8  NOT (a OR b)
        [1, -1, -1, 2],  # 9  NOT (a XOR b)
        [1, 0, -1, 0],   # 10 NOT b
        [1, 0, -1, 1],   # 11 a OR NOT b
        [1, -1, 0, 0],   # 12 NOT a
        [1, -1, 0, 1],   # 13 NOT a OR b
        [1, 0, 0, -1],   # 14 NOT (a AND b)
        [1, 0, 0, 0],    # 15 True
    ],
    dtype=np.float32,
)  # [16, 4]


# ---------------------------------------------------------------- device IR
def build_nc(B=B_TOT, IN=IN_DIM, OUT=OUT_PC, NJ=256, Q=2, HG=4, PSB=4, OSB=10, GB=4, UB=4, ROT="vsvs", SKEW=2):
    """Build the per-core Bass module (SPMD; all cores run the same IR)."""
    import sys

    if "/opt/trn_rl_repo" not in sys.path:
        sys.path.insert(0, "/opt/trn_rl_repo")

    import concourse.tile as tile
    from concourse import bacc, mybir
    from concourse.masks import make_identity
    from contextlib import ExitStack

    f32 = mybir.dt.float32
    f16 = mybir.dt.float16
    i16 = mybir.dt.int16
    BH = B // Q              # batch half (1024)
    PBH = BH // 128          # h-blocks per half (8)
    NCH = OUT // NJ          # j chunks per core (4)
    SLOTS = NJ // 128        # 128-wide j slots per chunk (2)
    RPT = OUT // 128         # j slots per core (8)
    IW = OUT // 16           # idx columns per tensor (64)
    MW = 2 * IW + 8 * RPT    # meta width in i16: idxa | idxb | wk-bits

    nc = bacc.Bacc("TRN2", target_bir_lowering=False)
    xT = nc.declare_dram_parameter("xT", [IN, B], f16, isOutput=False)
    meta = nc.declare_dram_parameter("meta16", [128, MW], i16, isOutput=False)
    out = nc.declare_dram_parameter("out", [B, OUT], f16, isOutput=True)

    Ident = mybir.ActivationFunctionType.Identity
    MULT = mybir.AluOpType.mult
    ADD = mybir.AluOpType.add

    with tile.TileContext(nc) as tc, ExitStack() as ctx:
        cpool = ctx.enter_context(tc.tile_pool(name="consts", bufs=1))
        mt = cpool.tile([128, MW], i16, name="mt")
        nc.sync.dma_start(mt[:, 0:2 * IW], meta[:, 0:2 * IW])
        nc.sync.dma_start(mt[:, 2 * IW:MW], meta[:, 2 * IW:MW])
        wk = cpool.tile([128, 4 * RPT], f32, name="wk")
        nc.vector.tensor_copy(wk[:], mt[:, 2 * IW:MW].bitcast(f32))
        identf = cpool.tile([128, 128], f32, name="identf")
        make_identity(nc, identf[:])
        ident = cpool.tile([128, 128], f16, name="ident")
        nc.vector.tensor_copy(ident[:], identf[:])

        gpool = ctx.enter_context(tc.tile_pool(name="gath", bufs=GB))
        upool = ctx.enter_context(tc.tile_pool(name="uvt", bufs=UB))
        psumO = ctx.enter_context(tc.tile_pool(name="psumO", bufs=PSB, space="PSUM"))
        ostg = ctx.enter_context(tc.tile_pool(name="ostg", bufs=OSB))

        NJ16 = NJ // 16
        # issue every gather up front so DMA engines stay saturated
        gab = {}
        for ck in range(NCH):
            for q in range(Q):
                ga = gpool.tile([128, SLOTS, BH], f16, tag="ga")
                nc.gpsimd.dma_gather(
                    ga[:], xT[:, q * BH:(q + 1) * BH],
                    mt[:, ck * NJ16:(ck + 1) * NJ16],
                    NJ, NJ, BH, elem_step=B,
                )
                gb = gpool.tile([128, SLOTS, BH], f16, tag="gb")
                nc.gpsimd.dma_gather(
                    gb[:], xT[:, q * BH:(q + 1) * BH],
                    mt[:, IW + ck * NJ16:IW + (ck + 1) * NJ16],
                    NJ, NJ, BH, elem_step=B,
                )
                gab[ck, q] = (ga, gb)

        ncp = 0  # copy-engine round robin counter

        def emit_drain(stage):
            """PE transpose-accumulate + PSUM drain + store for one stage.

            The transposes are REAL matmuls against the fp16 identity
            (lhsT.T @ I): fp16 is_transpose matmuls do not accumulate in
            PSUM on hardware, but ordinary fp16 matmuls accumulate in f32
            PSUM at the same 1 cycle/row.
            """
            nonlocal ncp
            ck, q, ts_t, ts_v = stage
            for hg in range(PBH // HG):
                og = ostg.tile([128, HG, NJ], f16, tag="og")
                po = psumO.tile([128, HG, NJ], f32, tag="po")
                for hi in range(HG):
                    hs = slice((hg * HG + hi) * 128, (hg * HG + hi + 1) * 128)
                    for c in range(SLOTS):
                        nc.tensor.matmul(
                            po[:, hi, c * 128:(c + 1) * 128],
                            ts_t[c][:, hs], ident[:],
                            start=True, stop=False,
                        )
                        nc.tensor.matmul(
                            po[:, hi, c * 128:(c + 1) * 128],
                            ts_v[c][:, hs], ident[:],
                            start=False, stop=True,
                        )
                # round-robin the PSUM drain (Pool cannot read PSUM on HW)
                eng = {"v": nc.vector, "s": nc.scalar}[ROT[ncp % len(ROT)]]
                ncp += 1
                if eng is nc.scalar:
                    eng.copy(og[:], po[:])
                else:
                    eng.tensor_copy(og[:], po[:])
                row0 = q * BH + hg * HG * 128
                nc.sync.dma_start(
                    out[row0:row0 + HG * 128, ck * NJ:(ck + 1) * NJ]
                    .rearrange("(i p) j -> p i j", p=128),
                    og[:],
                )

        # software-pipelined emission: stage k's drain section is emitted
        # after stage k+1's elementwise section so the PSUM-drain copies
        # don't head-of-line-block the next stage's v/t in the engine queues
        pend = []
        for ck in range(NCH):
            for q in range(Q):
                ga, gb = gab[ck, q]
                ts_v, ts_t = [], []
                for c in range(SLOTS):
                    r = ck * SLOTS + c
                    # u = W3*a + W2 on ACT (per-partition scalar scale/bias)
                    u = upool.tile([128, BH], f16, tag="u")
                    nc.scalar.activation(
                        u[:], ga[:, c, :], Ident,
                        scale=wk[:, 3 * RPT + r:3 * RPT + r + 1],
                        bias=wk[:, 2 * RPT + r:2 * RPT + r + 1],
                    )
                    # v = W1*a + W0 on DVE (all-fp16 -> 2x DVE mode)
                    v = upool.tile([128, BH], f16, tag="v")
                    nc.vector.tensor_scalar(
                        v[:], ga[:, c, :],
                        wk[:, 1 * RPT + r:1 * RPT + r + 1],
                        wk[:, 0 * RPT + r:0 * RPT + r + 1],
                        op0=MULT, op1=ADD,
                    )
                    # t = u * b on DVE (fp16 2x)
                    t = upool.tile([128, BH], f16, tag="t")
                    nc.vector.tensor_tensor(t[:], u[:], gb[:, c, :], op=MULT)
                    ts_v.append(v)
                    ts_t.append(t)
                pend.append((ck, q, ts_t, ts_v))
                if len(pend) > SKEW:
                    emit_drain(pend.pop(0))
        for st in pend:
            emit_drain(st)
    nc.compile()
    return nc


# ---------------------------------------------------------------- host side
def _wrap_idx(idx, OUT, NJ):
    """Pack an index vector into dma_gather's wrapped int16 layout.

    Per chunk ck the NJ indices live in columns [ck*NJ/16, (ck+1)*NJ/16):
    idx16[p, ck*NJ/16 + s] = idx[ck*NJ + s*16 + p%16], replicated over the
    8 groups of 16 partitions.
    """
    nch = OUT // NJ
    a = np.asarray(idx).astype(np.int16).reshape(nch, NJ // 16, 16)  # [ck, s, p]
    a = a.transpose(2, 0, 1).reshape(16, nch * (NJ // 16))           # [p, ck*s]
    return np.ascontiguousarray(np.tile(a, (8, 1)))                  # [128, ...]


def _prep_inputs(x, weights, idx_a, idx_b, NJ=256):
    x = np.asarray(x, dtype=np.float32)
    weights = np.asarray(weights, dtype=np.float32)
    # W = softmax(weights) @ C  -> [OUT, 4]
    e = np.exp(weights - weights.max(axis=-1, keepdims=True))
    W = (e / e.sum(axis=-1, keepdims=True)).astype(np.float32) @ GATE_C
    xT = np.ascontiguousarray(x.T.astype(np.float16))  # [IN, B], replicated
    RPT = OUT_PC // 128
    in_maps = []
    for core in range(NCORES):
        j0 = core * OUT_PC
        # wk4[q, m*RPT + r] = W[j0 + r*128 + q, m]
        Wc = W[j0:j0 + OUT_PC].reshape(RPT, 128, 4)       # [r, q, m]
        wk4 = np.ascontiguousarray(
            Wc.transpose(1, 2, 0).reshape(128, 4 * RPT)   # [q, m*RPT+r]
        )
        meta = np.concatenate(
            [
                _wrap_idx(idx_a[j0:j0 + OUT_PC], OUT_PC, NJ),
                _wrap_idx(idx_b[j0:j0 + OUT_PC], OUT_PC, NJ),
                wk4.view(np.int16),
            ],
            axis=1,
        )
        in_maps.append({"xT": xT, "meta16": np.ascontiguousarray(meta)})
    return in_maps


_NC_CACHE = {}


def _get_nc():
    if "nc" not in _NC_CACHE:
        _NC_CACHE["nc"] = build_nc()
    return _NC_CACHE["nc"]


def kernel(x, weights, idx_a, idx_b):
    import sys

    if "/opt/trn_rl_repo" not in sys.path:
        sys.path.insert(0, "/opt/trn_rl_repo")
    from concourse.bass_utils import run_bass_kernel_spmd

    nc = _get_nc()
    in_maps = _prep_inputs(x, weights, idx_a, idx_b)
    res = run_bass_kernel_spmd(nc, in_maps, list(range(NCORES)))
    return np.concatenate(
        [r["out"] for r in res.results], axis=1, dtype=np.float32
    )


if __name__ == "__main__":
    nc = build_nc()
    print("built OK")


# revision 4
# speedup vs baseline: 1.0713x; 1.0043x over previous
"""Trainium2 Bass kernel for nn_LogicLayer (differentiable logic-gate layer).

Reference computation:
    a = x[:, idx_a]; b = x[:, idx_b]                  # [B, OUT] gathers
    w = softmax(weights, -1)                          # [OUT, 16]
    out = sum_k w[:, k] * gate_k(a, b)

Every gate value is of the form c0 + c1*a + c2*b + c3*a*b, so
    out[i, j] = W0[j] + W1[j]*a + W2[j]*b + W3[j]*a*b
with W = softmax(weights) @ C, C the [16, 4] gate-coefficient table
(host-precomputed metadata, like the index packing).

Sharding: out_dim split 8 ways (1024 j's per core), batch unsharded.
Each core receives x pre-transposed to fp16 xT [IN, B] in DRAM, so the
kernel is a single gather->gates->transpose-back phase, 2D-pipelined
over (j-chunk, batch-half):
  1. dma_gather fp16 rows of xT (elem = one batch half) for idx_a/idx_b
     (j lands on partitions in (q, r) wrap: j = r*128 + q)
  2. u = W3*a + W2 (ACT), v = W1*a + W0 (DVE ts), t = u*b (DVE tt)
  3. out = t + v realized in PSUM fp16 by PE transpose-accumulate,
     copied to SBUF (DVE/ACT/Pool round-robin) and stored batched
     (4 h-blocks per DMA) in natural [B, OUT] layout.
Output is stored fp16 (well within the 2e-2 gate tolerance since all
gate math is fp16 anyway) and upcast to f32 on the host.
"""

import numpy as np

# ---------------------------------------------------------------- constants
B_TOT, IN_DIM, OUT_DIM = 2048, 8192, 8192
NCORES = 8
OUT_PC = OUT_DIM // NCORES   # 1024 j's per core

# value = c0 + c1*a + c2*b + c3*ab  for each of the 16 gates
GATE_C = np.array(
    [
        # c0  c1  c2  c3
        [0, 0, 0, 0],    # 0  False
        [0, 0, 0, 1],    # 1  a AND b
        [0, 1, 0, -1],   # 2  a AND NOT b
        [0, 1, 0, 0],    # 3  a
        [0, 0, 1, -1],   # 4  NOT a AND b
        [0, 0, 1, 0],    # 5  b
        [0, 1, 1, -2],   # 6  a XOR b
        [0, 1, 1, -1],   # 7  a OR b
        [1, -1, -1, 1],  # 8  NOT (a OR b)
        [1, -1, -1, 2],  # 9  NOT (a XOR b)
        [1, 0, -1, 0],   # 10 NOT b
        [1, 0, -1, 1],   # 11 a OR NOT b
        [1, -1, 0, 0],   # 12 NOT a
        [1, -1, 0, 1],   # 13 NOT a OR b
        [1, 0, 0, -1],   # 14 NOT (a AND b)
        [1, 0, 0, 0],    # 15 True
    ],
    dtype=np.float32,
)  # [16, 4]


# ---------------------------------------------------------------- device IR
def build_nc(B=B_TOT, IN=IN_DIM, OUT=OUT_PC, NJ=256, Q=2, HG=4, PSB=4, OSB=10, GB=4, UB=4, ROT="vsvs", SKEW=2):
    """Build the per-core Bass module (SPMD; all cores run the same IR)."""
    import sys

    if "/opt/trn_rl_repo" not in sys.path:
        sys.path.insert(0, "/opt/trn_rl_repo")

    import concourse.tile as tile
    from concourse import bacc, mybir
    from concourse.masks import make_identity
    from contextlib import ExitStack

    f32 = mybir.dt.float32
    f16 = mybir.dt.float16
    i16 = mybir.dt.int16
    BH = B // Q              # batch half (1024)
    PBH = BH // 128          # h-blocks per half (8)
    NCH = OUT // NJ          # j chunks per core (4)
    SLOTS = NJ // 128        # 128-wide j slots per chunk (2)
    RPT = OUT // 128         # j slots per core (8)
    IW = OUT // 16           # idx columns per tensor (64)
    MW = 2 * IW + 8 * RPT    # meta width in i16: idxa | idxb | wk-bits

    nc = bacc.Bacc("TRN2", target_bir_lowering=False)
    xT = nc.declare_dram_parameter("xT", [IN, B], f16, isOutput=False)
    meta = nc.declare_dram_parameter("meta16", [128, MW], i16, isOutput=False)
    out = nc.declare_dram_parameter("out", [B, OUT], f16, isOutput=True)

    Ident = mybir.ActivationFunctionType.Identity
    MULT = mybir.AluOpType.mult
    ADD = mybir.AluOpType.add

    with tile.TileContext(nc) as tc, ExitStack() as ctx:
        cpool = ctx.enter_context(tc.tile_pool(name="consts", bufs=1))
        mt = cpool.tile([128, MW], i16, name="mt")
        NJ16_ = NJ // 16
        nc.sync.dma_start(mt[:, 0:NJ16_], meta[:, 0:NJ16_])
        nc.sync.dma_start(mt[:, NJ16_:2 * IW], meta[:, NJ16_:2 * IW])
        nc.sync.dma_start(mt[:, 2 * IW:MW], meta[:, 2 * IW:MW])
        wk = cpool.tile([128, 4 * RPT], f32, name="wk")
        nc.vector.tensor_copy(wk[:], mt[:, 2 * IW:MW].bitcast(f32))
        identf = cpool.tile([128, 128], f32, name="identf")
        make_identity(nc, identf[:])
        ident = cpool.tile([128, 128], f16, name="ident")
        nc.vector.tensor_copy(ident[:], identf[:])

        gpool = ctx.enter_context(tc.tile_pool(name="gath", bufs=GB))
        upool = ctx.enter_context(tc.tile_pool(name="uvt", bufs=UB))
        psumO = ctx.enter_context(tc.tile_pool(name="psumO", bufs=PSB, space="PSUM"))
        ostg = ctx.enter_context(tc.tile_pool(name="ostg", bufs=OSB))

        NJ16 = NJ // 16
        # issue every gather up front so DMA engines stay saturated
        gab = {}
        for ck in range(NCH):
            for q in range(Q):
                ga = gpool.tile([128, SLOTS, BH], f16, tag="ga")
                nc.gpsimd.dma_gather(
                    ga[:], xT[:, q * BH:(q + 1) * BH],
                    mt[:, ck * NJ16:(ck + 1) * NJ16],
                    NJ, NJ, BH, elem_step=B,
                )
                gb = gpool.tile([128, SLOTS, BH], f16, tag="gb")
                nc.gpsimd.dma_gather(
                    gb[:], xT[:, q * BH:(q + 1) * BH],
                    mt[:, IW + ck * NJ16:IW + (ck + 1) * NJ16],
                    NJ, NJ, BH, elem_step=B,
                )
                gab[ck, q] = (ga, gb)

        ncp = 0  # copy-engine round robin counter

        def emit_drain(stage):
            """PE transpose-accumulate + PSUM drain + store for one stage.

            The transposes are REAL matmuls against the fp16 identity
            (lhsT.T @ I): fp16 is_transpose matmuls do not accumulate in
            PSUM on hardware, but ordinary fp16 matmuls accumulate in f32
            PSUM at the same 1 cycle/row.
            """
            nonlocal ncp
            ck, q, ts_t, ts_v = stage
            for hg in range(PBH // HG):
                og = ostg.tile([128, HG, NJ], f16, tag="og")
                po = psumO.tile([128, HG, NJ], f32, tag="po")
                for hi in range(HG):
                    hs = slice((hg * HG + hi) * 128, (hg * HG + hi + 1) * 128)
                    for c in range(SLOTS):
                        nc.tensor.matmul(
                            po[:, hi, c * 128:(c + 1) * 128],
                            ts_t[c][:, hs], ident[:],
                            start=True, stop=False,
                        )
                        nc.tensor.matmul(
                            po[:, hi, c * 128:(c + 1) * 128],
                            ts_v[c][:, hs], ident[:],
                            start=False, stop=True,
                        )
                # round-robin the PSUM drain (Pool cannot read PSUM on HW)
                eng = {"v": nc.vector, "s": nc.scalar}[ROT[ncp % len(ROT)]]
                ncp += 1
                if eng is nc.scalar:
                    eng.copy(og[:], po[:])
                else:
                    eng.tensor_copy(og[:], po[:])
                row0 = q * BH + hg * HG * 128
                nc.sync.dma_start(
                    out[row0:row0 + HG * 128, ck * NJ:(ck + 1) * NJ]
                    .rearrange("(i p) j -> p i j", p=128),
                    og[:],
                )

        # software-pipelined emission: stage k's drain section is emitted
        # after stage k+1's elementwise section so the PSUM-drain copies
        # don't head-of-line-block the next stage's v/t in the engine queues
        pend = []
        for ck in range(NCH):
            for q in range(Q):
                ga, gb = gab[ck, q]
                ts_v, ts_t = [], []
                for c in range(SLOTS):
                    r = ck * SLOTS + c
                    # u = W3*a + W2 on ACT (per-partition scalar scale/bias)
                    u = upool.tile([128, BH], f16, tag="u")
                    nc.scalar.activation(
                        u[:], ga[:, c, :], Ident,
                        scale=wk[:, 3 * RPT + r:3 * RPT + r + 1],
                        bias=wk[:, 2 * RPT + r:2 * RPT + r + 1],
                    )
                    # v = W1*a + W0 on DVE (all-fp16 -> 2x DVE mode)
                    v = upool.tile([128, BH], f16, tag="v")
                    nc.vector.tensor_scalar(
                        v[:], ga[:, c, :],
                        wk[:, 1 * RPT + r:1 * RPT + r + 1],
                        wk[:, 0 * RPT + r:0 * RPT + r + 1],
                        op0=MULT, op1=ADD,
                    )
                    # t = u * b on DVE (fp16 2x)
                    t = upool.tile([128, BH], f16, tag="t")
                    nc.vector.tensor_tensor(t[:], u[:], gb[:, c, :], op=MULT)
                    ts_v.append(v)
                    ts_t.append(t)
                pend.append((ck, q, ts_t, ts_v))
                if len(pend) > SKEW:
                    emit_drain(pend.pop(0))
        for st in pend:
            emit_drain(st)
    nc.compile()
    return nc


# ---------------------------------------------------------------- host side
def _wrap_idx(idx, OUT, NJ):
    """Pack an index vector into dma_gather's wrapped int16 layout.

    Per chunk ck the NJ indices live in columns [ck*NJ/16, (ck+1)*NJ/16):
    idx16[p, ck*NJ/16 + s] = idx[ck*NJ + s*16 + p%16], replicated over the
    8 groups of 16 partitions.
    """
    nch = OUT // NJ
    a = np.asarray(idx).astype(np.int16).reshape(nch, NJ // 16, 16)  # [ck, s, p]
    a = a.transpose(2, 0, 1).reshape(16, nch * (NJ // 16))           # [p, ck*s]
    return np.ascontiguousarray(np.tile(a, (8, 1)))                  # [128, ...]


def _prep_inputs(x, weights, idx_a, idx_b, NJ=256):
    x = np.asarray(x, dtype=np.float32)
    weights = np.asarray(weights, dtype=np.float32)
    # W = softmax(weights) @ C  -> [OUT, 4]
    e = np.exp(weights - weights.max(axis=-1, keepdims=True))
    W = (e / e.sum(axis=-1, keepdims=True)).astype(np.float32) @ GATE_C
    xT = np.ascontiguousarray(x.T.astype(np.float16))  # [IN, B], replicated
    RPT = OUT_PC // 128
    in_maps = []
    for core in range(NCORES):
        j0 = core * OUT_PC
        # wk4[q, m*RPT + r] = W[j0 + r*128 + q, m]
        Wc = W[j0:j0 + OUT_PC].reshape(RPT, 128, 4)       # [r, q, m]
        wk4 = np.ascontiguousarray(
            Wc.transpose(1, 2, 0).reshape(128, 4 * RPT)   # [q, m*RPT+r]
        )
        meta = np.concatenate(
            [
                _wrap_idx(idx_a[j0:j0 + OUT_PC], OUT_PC, NJ),
                _wrap_idx(idx_b[j0:j0 + OUT_PC], OUT_PC, NJ),
                wk4.view(np.int16),
            ],
            axis=1,
        )
        in_maps.append({"xT": xT, "meta16": np.ascontiguousarray(meta)})
    return in_maps


_NC_CACHE = {}


def _get_nc():
    if "nc" not in _NC_CACHE:
        _NC_CACHE["nc"] = build_nc()
    return _NC_CACHE["nc"]


def kernel(x, weights, idx_a, idx_b):
    import sys

    if "/opt/trn_rl_repo" not in sys.path:
        sys.path.insert(0, "/opt/trn_rl_repo")
    from concourse.bass_utils import run_bass_kernel_spmd

    nc = _get_nc()
    in_maps = _prep_inputs(x, weights, idx_a, idx_b)
    res = run_bass_kernel_spmd(nc, in_maps, list(range(NCORES)))
    return np.concatenate(
        [r["out"] for r in res.results], axis=1, dtype=np.float32
    )


if __name__ == "__main__":
    nc = build_nc()
    print("built OK")
